# revision 4
# baseline (speedup 1.0000x reference)
"""Trainium2 Bass kernel for nn_DQN CEM sampling problem.

Data-parallel over batch: 4096 rows -> 8 cores x 512 rows. Each core runs the
full 99-step CEM loop on its shard; the tiny MLP weights are replicated.

Per-core layout (B=512 rows as G=4 groups of P=128 partitions):
  - MLP runs feature-major: activations [feature, n] with n = (g, p, m) flattened,
    so each layer is a single tensor-engine pass (K on partitions).
  - q is produced batch-major-compatible via 4-way column-tiled matmuls
    (tile_position=(0,32g)), drained to SBUF, then "flipped" (partition<->free)
    with strided SBUF->SBUF DMAs into a [128, 4, 64] tile (cols 50:64 = -1e30 pad).
  - top-32 of 50 per row via a tie-exact bitonic network (sort both 32-halves
    descending, then one compare of half A vs reversed half B) on the vector
    engine; mean/var via bn_stats/bn_aggr; std = sqrt(var * 32/31) (ddof=1).
  - next angles = mu + std * eps sampled batch-major, flipped back into the
    feature-major x row via DMA.

All PRNG tensors (angles0, eps_t) are host-precomputed with the exact jax calls
the reference makes (key 42), laid out per-core, and streamed from DRAM.
"""

import numpy as np

BATCH = 4096
M = 50
NTOP = 32
ITERS = 100  # reference ITERS; device runs ITERS-1 = 99 qnet/stats steps
HIDDEN = 100
NCORES = 8
B = BATCH // NCORES  # 512 rows per core
G = 4                # partition groups per core
P = 128              # rows per group (partitions)
NPG = P * M          # columns per group = 6400
N = G * NPG          # columns per core = 25600
NEG = -1.0e30
TWO_PI = 6.283185307179586

_PROG_CACHE = {}


def _mlp_tiles():
    """(ti, w) matmul column tiles covering one group's NPG columns."""
    tiles = []
    off = 0
    while off < NPG:
        w = min(512, NPG - off)
        tiles.append((off, w))
        off += w
    return tiles


def build_program(n_steps=ITERS - 1):
    """Build the single-core Bass/Tile program (SPMD across cores).

    n_steps = number of qnet+stats iterations (reference does 99)."""
    import concourse.bacc as bacc
    import concourse.bass as bass
    import concourse.tile as tile
    import concourse.mybir as mybir

    f32 = mybir.dt.float32
    Alu = mybir.AluOpType
    Act = mybir.ActivationFunctionType

    nc = bacc.Bacc("TRN2", target_bir_lowering=False, debug=False)

    # DRAM I/O
    XREP = nc.dram_tensor("XREP", [2, N], f32, kind="ExternalInput")
    A0 = nc.dram_tensor("A0", [N], f32, kind="ExternalInput")
    EPS = nc.dram_tensor("EPS", [max(n_steps - 1, 1), P, G * M], f32,
                         kind="ExternalInput")
    W1D = nc.dram_tensor("W1D", [3, HIDDEN], f32, kind="ExternalInput")
    W2D = nc.dram_tensor("W2D", [HIDDEN, HIDDEN], f32, kind="ExternalInput")
    W3D = nc.dram_tensor("W3D", [HIDDEN, 32], f32, kind="ExternalInput")
    B1D = nc.dram_tensor("B1D", [HIDDEN], f32, kind="ExternalInput")
    B2D = nc.dram_tensor("B2D", [HIDDEN], f32, kind="ExternalInput")
    B3D = nc.dram_tensor("B3D", [P], f32, kind="ExternalInput")
    OUT = nc.dram_tensor("OUT", [B], f32, kind="ExternalOutput")

    tiles = _mlp_tiles()

    with tile.TileContext(nc) as tc:
        with (
            tc.tile_pool(name="statics", bufs=1) as statics,
            tc.tile_pool(name="h1pool", bufs=3) as h1pool,
            tc.tile_pool(name="h2pool", bufs=3) as h2pool,
            tc.tile_pool(name="ps1", bufs=2, space=bass.MemorySpace.PSUM) as ps1,
            tc.tile_pool(name="ps2", bufs=2, space=bass.MemorySpace.PSUM) as ps2,
            tc.tile_pool(name="psq", bufs=2, space=bass.MemorySpace.PSUM) as psq,
        ):
            # --- static tiles ---
            x = statics.tile([3, N], f32)             # fm input rows s0,s1,angle
            q_sb = statics.tile([P, NPG], f32)        # q rows at partitions 0/32/64/96
            q64 = statics.tile([P, G, 64], f32)       # batch-major q + pad
            SA = statics.tile([P, G * 64], f32)       # sort ping
            SB = statics.tile([P, G * 64], f32)       # sort pong
            top32 = statics.tile([P, G, NTOP], f32)
            bnst = statics.tile([P, G, 6], f32)
            mv = statics.tile([P, G, 2], f32)
            mu = statics.tile([P, G], f32)
            std = statics.tile([P, G], f32)
            a_bm = statics.tile([P, G, M], f32)        # sampled angles, batch-major
            tmp_s = statics.tile([P, G, M], f32)
            eps_sb = statics.tile([P, G * M], f32)
            out_sb = statics.tile([P, G], f32)
            w1s = statics.tile([3, HIDDEN], f32)
            w2s = statics.tile([HIDDEN, HIDDEN], f32)
            w3s = statics.tile([HIDDEN, 32], f32)
            b1s = statics.tile([HIDDEN, 1], f32)
            b2s = statics.tile([HIDDEN, 1], f32)
            b3s = statics.tile([P, 1], f32)

            # --- one-time setup ---
            nc.sync.dma_start(out=w1s, in_=W1D.ap())
            nc.sync.dma_start(out=w2s, in_=W2D.ap())
            nc.sync.dma_start(out=w3s, in_=W3D.ap())
            nc.sync.dma_start(out=b1s, in_=B1D.ap())
            nc.sync.dma_start(out=b2s, in_=B2D.ap())
            nc.sync.dma_start(out=b3s, in_=B3D.ap())
            nc.sync.dma_start(out=x[0:2, :], in_=XREP.ap())
            nc.vector.memset(q64[:, :, M:64], NEG)

            def mlp_and_q():
                """x row2 -> q_sb (all 4 groups), feature-major MLP."""
                for (off, w) in tiles:
                    qp = psq.tile([P, 512], f32, tag="qp")
                    for g in range(G):
                        col = g * NPG + off
                        h1p = ps1.tile([HIDDEN, 512], f32, tag="h1p")
                        nc.tensor.matmul(h1p[:, :w], w1s, x[:, col:col + w])
                        h1s = h1pool.tile([HIDDEN, 512], f32, tag="h1s")
                        nc.scalar.activation(h1s[:, :w], h1p[:, :w], Act.Relu,
                                             bias=b1s, scale=1.0)
                        h2p = ps2.tile([HIDDEN, 512], f32, tag="h2p")
                        nc.tensor.matmul(h2p[:, :w], w2s, h1s[:, :w])
                        h2s = h2pool.tile([HIDDEN, 512], f32, tag="h2s")
                        nc.vector.tensor_scalar(h2s[:, :w], h2p[:, :w],
                                                scalar1=b2s, scalar2=0.0,
                                                op0=Alu.add, op1=Alu.max)
                        nc.tensor.matmul(qp[32 * g:32 * g + 32, :w], w3s,
                                         h2s[:, :w], tile_position=(0, 32 * g))
                    # drain all 4 groups' q rows (plus zero rows) in one op
                    nc.scalar.copy(q_sb[:, off:off + w], qp[:, :w])

            def q_flip():
                for g in range(G):
                    nc.sync.dma_start(
                        out=q64[:, g, 0:M],
                        in_=q_sb[32 * g:32 * g + 1, :].rearrange(
                            "a (p m) -> a p m", m=M),
                    )

            def a_flip():
                for g in range(G):
                    nc.sync.dma_start(
                        out=x[2:3, g * NPG:(g + 1) * NPG].rearrange(
                            "a (p m) -> a p m", m=M),
                        in_=a_bm[:, g, :],
                    )

            def sort_stats():
                """q64 -> top32 -> mv (mean, var) -> mu (incl. b3)."""
                # halves view: [P, 8, 32] on each buffer
                def hv(t):
                    return t.rearrange("p (h m) -> p h m", m=32)

                src = hv(q64.rearrange("p g m -> p (g m)"))
                dst_list = [hv(SA), hv(SB)]
                which = 0
                for k in [2, 4, 8, 16, 32]:
                    # flip substage on blocks of k
                    dst = dst_list[which]; which ^= 1
                    nbk = 32 // k
                    s4 = src.rearrange("p h (nb k) -> p h nb k", k=k)
                    d4 = dst.rearrange("p h (nb k) -> p h nb k", k=k)
                    lo_in = s4[:, :, :, 0:k // 2]
                    hi_in = s4[:, :, :, k // 2:k]
                    lo_rev = s4[:, :, :, k // 2 - 1::-1] if k > 1 else None
                    hi_rev = s4[:, :, :, k - 1:k // 2 - 1:-1]
                    nc.vector.tensor_tensor(d4[:, :, :, 0:k // 2], lo_in,
                                            hi_rev, op=Alu.max)
                    nc.vector.tensor_tensor(d4[:, :, :, k // 2:k], hi_in,
                                            lo_rev, op=Alu.min)
                    src = dst
                    d = k // 4
                    while d >= 1:
                        dst = dst_list[which]; which ^= 1
                        s5 = src.rearrange("p h (nb two d) -> p h nb two d",
                                           two=2, d=d)
                        d5 = dst.rearrange("p h (nb two d) -> p h nb two d",
                                           two=2, d=d)
                        nc.vector.tensor_tensor(d5[:, :, :, 0, :],
                                                s5[:, :, :, 0, :],
                                                s5[:, :, :, 1, :], op=Alu.max)
                        nc.vector.tensor_tensor(d5[:, :, :, 1, :],
                                                s5[:, :, :, 0, :],
                                                s5[:, :, :, 1, :], op=Alu.min)
                        src = dst
                        d //= 2
                # merge: top32[p,g,i] = max(A[p,g,i], B[p,g,31-i])
                sg = src.rearrange("p (g h) m -> p g h m", g=G)
                nc.vector.tensor_tensor(top32, sg[:, :, 0, :],
                                        sg[:, :, 1, ::-1], op=Alu.max)
                for g in range(G):
                    nc.vector.bn_stats(bnst[:, g, :], top32[:, g, :])
                    nc.vector.bn_aggr(mv[:, g, :], bnst[:, g:g + 1, :])
                nc.vector.tensor_scalar(mu, mv[:, :, 0], scalar1=b3s,
                                        scalar2=None, op0=Alu.add)

            def sample(t):
                """a_bm = mu + std * eps[t]  (t is python int or loop var)."""
                nc.scalar.activation(std, mv[:, :, 1], Act.Sqrt,
                                     scale=float(NTOP) / (NTOP - 1))
                if isinstance(t, int):
                    eps_src = EPS.ap()[t - 1:t, :, :]
                else:
                    eps_src = EPS.ap()[bass.ds(t - 1, 1), :, :]
                nc.sync.dma_start(out=eps_sb, in_=eps_src)
                epsv = eps_sb.rearrange("p (g m) -> p g m", m=M)
                stdb = std.unsqueeze(2).to_broadcast((P, G, M))
                mub = mu.unsqueeze(2).to_broadcast((P, G, M))
                nc.vector.tensor_tensor(tmp_s, epsv, stdb, op=Alu.mult)
                nc.vector.tensor_tensor(a_bm, tmp_s, mub, op=Alu.add)

            # ---- t = 0 ----
            nc.sync.dma_start(out=x[2:3, :], in_=A0.ap().rearrange("(a n) -> a n", a=1))
            mlp_and_q()
            q_flip()
            sort_stats()
            if n_steps > 1:
                sample(1)

            # ---- t = 1 .. n_steps-2 (dynamic loop) ----
            if n_steps > 2:
                with tc.For_i(1, n_steps - 1) as it:
                    a_flip()
                    mlp_and_q()
                    q_flip()
                    sort_stats()
                    sample(it + 1)

            # ---- t = n_steps-1 (final) ----
            if n_steps > 1:
                a_flip()
                mlp_and_q()
                q_flip()
                sort_stats()
            nc.vector.tensor_scalar(out_sb, mu, scalar1=TWO_PI, scalar2=None,
                                    op0=Alu.mult)
            nc.sync.dma_start(out=OUT.ap().rearrange("(g p) -> p g", p=P),
                              in_=out_sb)

    nc.compile()
    return nc


def host_prng(n_steps=ITERS - 1):
    """Exactly the reference's PRNG stream, on host CPU."""
    import jax
    import jax.numpy as jnp
    cpu = jax.devices("cpu")[0]
    with jax.default_device(cpu):
        # commit the key to CPU so the whole PRNG stream is computed by the
        # CPU backend bit-exactly (matching the reference harness)
        key = jax.device_put(jax.random.key(42), cpu)
        k0, kloop = jax.random.split(key)
        angles0 = np.asarray(jax.random.uniform(k0, (BATCH, M),
                                                dtype=jnp.float32))
        keys = jax.random.split(kloop, ITERS - 1)
        eps = np.stack([
            np.asarray(jax.random.normal(keys[t], (BATCH, M),
                                         dtype=jnp.float32))
            for t in range(max(n_steps - 1, 1))
        ])
    return angles0, eps


def make_in_map(core, states, W1, b1, W2, b2, W3, b3, angles0, eps):
    sl = slice(core * B, (core + 1) * B)
    S = np.ascontiguousarray(states[sl]).reshape(G, P, 2)
    xrep = np.ascontiguousarray(
        np.broadcast_to(S[:, :, None, :], (G, P, M, 2)).transpose(3, 0, 1, 2)
    ).reshape(2, N)
    a0 = np.ascontiguousarray(angles0[sl]).reshape(N)
    epsc = np.ascontiguousarray(
        eps[:, sl, :].reshape(-1, G, P, M).transpose(0, 2, 1, 3)
    ).reshape(-1, P, G * M)
    w3p = np.zeros((HIDDEN, 32), np.float32)
    w3p[:, 0] = W3[:, 0]
    return {
        "XREP": xrep.astype(np.float32),
        "A0": a0.astype(np.float32),
        "EPS": epsc.astype(np.float32),
        "W1D": W1.astype(np.float32),
        "W2D": W2.astype(np.float32),
        "W3D": w3p,
        "B1D": b1.astype(np.float32),
        "B2D": b2.astype(np.float32),
        "B3D": np.full((P,), np.float32(b3[0]), np.float32),
    }


LAST_RESULTS = None


def kernel(states, W1, b1, W2, b2, W3, b3, _trace=False):
    global LAST_RESULTS
    from concourse.bass_utils import run_bass_kernel_spmd

    n_steps = ITERS - 1
    if n_steps not in _PROG_CACHE:
        _PROG_CACHE[n_steps] = build_program(n_steps)
    nc = _PROG_CACHE[n_steps]

    angles0, eps = host_prng(n_steps)
    in_maps = [
        make_in_map(c, states, W1, b1, W2, b2, W3, b3, angles0, eps)
        for c in range(NCORES)
    ]
    res = run_bass_kernel_spmd(nc, in_maps, core_ids=list(range(NCORES)),
                               trace=_trace)
    LAST_RESULTS = res
    out = np.concatenate([res.results[c]["OUT"] for c in range(NCORES)])
    return out.astype(np.float32)


# revision 6
# speedup vs baseline: 2.2142x; 2.2142x over previous
"""Trainium2 Bass kernel for nn_DQN CEM sampling problem.

Data-parallel over batch: 4096 rows -> 8 cores x 512 rows. Each core runs the
full 99-step CEM loop on its shard; the tiny MLP weights are replicated.

Per-core layout (B=512 rows as G=4 groups of P=128 partitions):
  - MLP runs feature-major: activations [feature, n] with n = (g, p, m) flattened,
    so each layer is a single tensor-engine pass (K on partitions).
  - q is produced batch-major-compatible via 4-way column-tiled matmuls
    (tile_position=(0,32g)), drained to SBUF, then "flipped" (partition<->free)
    with strided SBUF->SBUF DMAs into a [128, 4, 64] tile (cols 50:64 = -1e30 pad).
  - top-32 of 50 per row via a tie-exact bitonic network (sort both 32-halves
    descending, then one compare of half A vs reversed half B) on the vector
    engine; mean/var via bn_stats/bn_aggr; std = sqrt(var * 32/31) (ddof=1).
  - next angles = mu + std * eps sampled batch-major, flipped back into the
    feature-major x row via DMA.

All PRNG tensors (angles0, eps_t) are host-precomputed with the exact jax calls
the reference makes (key 42), laid out per-core, and streamed from DRAM.
"""

import numpy as np

BATCH = 4096
M = 50
NTOP = 32
ITERS = 100  # reference ITERS; device runs ITERS-1 = 99 qnet/stats steps
HIDDEN = 100
NCORES = 8
B = BATCH // NCORES  # 512 rows per core
G = 4                # partition groups per core
P = 128              # rows per group (partitions)
NPG = P * M          # columns per group = 6400
N = G * NPG          # columns per core = 25600
NEG = -1.0e30
TWO_PI = 6.283185307179586

_PROG_CACHE = {}


def _mlp_tiles():
    """(ti, w) matmul column tiles covering one group's NPG columns."""
    tiles = []
    off = 0
    while off < NPG:
        w = min(512, NPG - off)
        tiles.append((off, w))
        off += w
    return tiles


def build_program(n_steps=ITERS - 1):
    """Build the single-core Bass/Tile program (SPMD across cores).

    n_steps = number of qnet+stats iterations (reference does 99)."""
    import concourse.bacc as bacc
    import concourse.bass as bass
    import concourse.tile as tile
    import concourse.mybir as mybir

    f32 = mybir.dt.float32
    f32r = mybir.dt.float32r
    Alu = mybir.AluOpType
    Act = mybir.ActivationFunctionType

    nc = bacc.Bacc("TRN2", target_bir_lowering=False, debug=False)

    # DRAM I/O
    XREP = nc.dram_tensor("XREP", [2, N], f32r, kind="ExternalInput")
    A0 = nc.dram_tensor("A0", [N], f32r, kind="ExternalInput")
    EPS = nc.dram_tensor("EPS", [max(n_steps - 1, 1), P, G * M], f32,
                         kind="ExternalInput")
    W1D = nc.dram_tensor("W1D", [3, HIDDEN], f32r, kind="ExternalInput")
    W2D = nc.dram_tensor("W2D", [HIDDEN, HIDDEN], f32r, kind="ExternalInput")
    W3D = nc.dram_tensor("W3D", [HIDDEN, 32], mybir.dt.float16, kind="ExternalInput")
    B1D = nc.dram_tensor("B1D", [HIDDEN], f32, kind="ExternalInput")
    B2D = nc.dram_tensor("B2D", [HIDDEN], f32, kind="ExternalInput")
    B3D = nc.dram_tensor("B3D", [P], f32, kind="ExternalInput")
    OUT = nc.dram_tensor("OUT", [B], f32, kind="ExternalOutput")

    tiles = _mlp_tiles()

    with tile.TileContext(nc) as tc:
        with (
            tc.tile_pool(name="statics", bufs=1) as statics,
            tc.tile_pool(name="h1pool", bufs=3) as h1pool,
            tc.tile_pool(name="h2pool", bufs=3) as h2pool,
            tc.tile_pool(name="ps1", bufs=2, space=bass.MemorySpace.PSUM) as ps1,
            tc.tile_pool(name="ps2", bufs=2, space=bass.MemorySpace.PSUM) as ps2,
            tc.tile_pool(name="psq", bufs=2, space=bass.MemorySpace.PSUM) as psq,
        ):
            # --- static tiles ---
            x = statics.tile([3, N], f32r)             # fm input rows s0,s1,angle
            q_sb = statics.tile([P, NPG], f32)        # q rows at partitions 0/32/64/96
            q64 = statics.tile([P, G, 64], f32)       # batch-major q + pad
            SA = statics.tile([P, G * 64], f32)       # sort ping
            SB = statics.tile([P, G * 64], f32)       # sort pong
            top32 = statics.tile([P, G, NTOP], f32)
            bnst = statics.tile([P, G, 6], f32)
            mv = statics.tile([P, G, 2], f32)
            mu = statics.tile([P, G], f32)
            std = statics.tile([P, G], f32)
            a_bm = statics.tile([P, G, M], f32r)        # sampled angles, batch-major
            tmp_s = statics.tile([P, G, M], f32)
            eps_sb = statics.tile([P, G * M], f32)
            out_sb = statics.tile([P, G], f32)
            w1s = statics.tile([3, HIDDEN], f32r)
            w2s = statics.tile([HIDDEN, HIDDEN], f32r)
            w3s = statics.tile([HIDDEN, 32], mybir.dt.float16)
            b1s = statics.tile([HIDDEN, 1], f32)
            b2s = statics.tile([HIDDEN, 1], f32)
            b3s = statics.tile([P, 1], f32)

            # --- one-time setup ---
            nc.sync.dma_start(out=w1s, in_=W1D.ap())
            nc.sync.dma_start(out=w2s, in_=W2D.ap())
            nc.sync.dma_start(out=w3s, in_=W3D.ap())
            nc.sync.dma_start(out=b1s, in_=B1D.ap())
            nc.sync.dma_start(out=b2s, in_=B2D.ap())
            nc.sync.dma_start(out=b3s, in_=B3D.ap())
            nc.sync.dma_start(out=x[0:2, :], in_=XREP.ap())
            nc.vector.memset(q64[:, :, M:64], NEG)

            def mlp_and_q():
                """x row2 -> q_sb (all 4 groups), feature-major MLP."""
                for (off, w) in tiles:
                    qp = psq.tile([P, 512], f32, tag="qp")
                    for g in range(G):
                        col = g * NPG + off
                        h1p = ps1.tile([HIDDEN, 512], f32, tag="h1p")
                        nc.tensor.matmul(h1p[:, :w], w1s, x[:, col:col + w])
                        h1s = h1pool.tile([HIDDEN, 512], f32r, tag="h1s")
                        nc.scalar.activation(h1s[:, :w], h1p[:, :w], Act.Relu,
                                             bias=b1s, scale=1.0)
                        h2p = ps2.tile([HIDDEN, 512], f32, tag="h2p")
                        nc.tensor.matmul(h2p[:, :w], w2s, h1s[:, :w])
                        h2s = h2pool.tile([HIDDEN, 512], mybir.dt.float16, tag="h2s")
                        nc.vector.tensor_scalar(h2s[:, :w], h2p[:, :w],
                                                scalar1=b2s, scalar2=0.0,
                                                op0=Alu.add, op1=Alu.max)
                        nc.tensor.matmul(qp[32 * g:32 * g + 32, :w], w3s,
                                         h2s[:, :w], tile_position=(0, 32 * g))
                    # drain all 4 groups' q rows (plus zero rows) in one op
                    nc.scalar.copy(q_sb[:, off:off + w], qp[:, :w])

            def q_flip():
                for g in range(G):
                    nc.sync.dma_start(
                        out=q64[:, g, 0:M],
                        in_=q_sb[32 * g:32 * g + 1, :].rearrange(
                            "a (p m) -> a p m", m=M),
                    )

            def a_flip():
                for g in range(G):
                    nc.sync.dma_start(
                        out=x[2:3, g * NPG:(g + 1) * NPG].rearrange(
                            "a (p m) -> a p m", m=M),
                        in_=a_bm[:, g, :],
                    )

            def sort_stats():
                """q64 -> top32 -> mv (mean, var) -> mu (incl. b3)."""
                # halves view: [P, 8, 32] on each buffer
                def hv(t):
                    return t.rearrange("p (h m) -> p h m", m=32)

                src = hv(q64.rearrange("p g m -> p (g m)"))
                dst_list = [hv(SA), hv(SB)]
                which = 0
                for k in [2, 4, 8, 16, 32]:
                    # flip substage on blocks of k
                    dst = dst_list[which]; which ^= 1
                    nbk = 32 // k
                    s4 = src.rearrange("p h (nb k) -> p h nb k", k=k)
                    d4 = dst.rearrange("p h (nb k) -> p h nb k", k=k)
                    lo_in = s4[:, :, :, 0:k // 2]
                    hi_in = s4[:, :, :, k // 2:k]
                    lo_rev = s4[:, :, :, k // 2 - 1::-1] if k > 1 else None
                    hi_rev = s4[:, :, :, k - 1:k // 2 - 1:-1]
                    nc.vector.tensor_tensor(d4[:, :, :, 0:k // 2], lo_in,
                                            hi_rev, op=Alu.max)
                    nc.vector.tensor_tensor(d4[:, :, :, k // 2:k], hi_in,
                                            lo_rev, op=Alu.min)
                    src = dst
                    d = k // 4
                    while d >= 1:
                        dst = dst_list[which]; which ^= 1
                        s5 = src.rearrange("p h (nb two d) -> p h nb two d",
                                           two=2, d=d)
                        d5 = dst.rearrange("p h (nb two d) -> p h nb two d",
                                           two=2, d=d)
                        nc.vector.tensor_tensor(d5[:, :, :, 0, :],
                                                s5[:, :, :, 0, :],
                                                s5[:, :, :, 1, :], op=Alu.max)
                        nc.vector.tensor_tensor(d5[:, :, :, 1, :],
                                                s5[:, :, :, 0, :],
                                                s5[:, :, :, 1, :], op=Alu.min)
                        src = dst
                        d //= 2
                # merge: top32[p,g,i] = max(A[p,g,i], B[p,g,31-i])
                sg = src.rearrange("p (g h) m -> p g h m", g=G)
                nc.vector.tensor_tensor(top32, sg[:, :, 0, :],
                                        sg[:, :, 1, ::-1], op=Alu.max)
                for g in range(G):
                    nc.vector.bn_stats(bnst[:, g, :], top32[:, g, :])
                    nc.vector.bn_aggr(mv[:, g, :], bnst[:, g:g + 1, :])
                nc.vector.tensor_scalar(mu, mv[:, :, 0], scalar1=b3s,
                                        scalar2=None, op0=Alu.add)

            def sample(t):
                """a_bm = mu + std * eps[t]  (t is python int or loop var)."""
                nc.scalar.activation(std, mv[:, :, 1], Act.Sqrt,
                                     scale=float(NTOP) / (NTOP - 1))
                if isinstance(t, int):
                    eps_src = EPS.ap()[t - 1:t, :, :]
                else:
                    eps_src = EPS.ap()[bass.ds(t - 1, 1), :, :]
                nc.sync.dma_start(out=eps_sb, in_=eps_src)
                epsv = eps_sb.rearrange("p (g m) -> p g m", m=M)
                stdb = std.unsqueeze(2).to_broadcast((P, G, M))
                mub = mu.unsqueeze(2).to_broadcast((P, G, M))
                nc.vector.tensor_tensor(tmp_s, epsv, stdb, op=Alu.mult)
                nc.vector.tensor_tensor(a_bm, tmp_s, mub, op=Alu.add)

            # ---- t = 0 ----
            nc.sync.dma_start(out=x[2:3, :], in_=A0.ap().rearrange("(a n) -> a n", a=1))
            mlp_and_q()
            q_flip()
            sort_stats()
            if n_steps > 1:
                sample(1)

            # ---- t = 1 .. n_steps-2 (dynamic loop) ----
            if n_steps > 2:
                with tc.For_i(1, n_steps - 1) as it:
                    a_flip()
                    mlp_and_q()
                    q_flip()
                    sort_stats()
                    sample(it + 1)

            # ---- t = n_steps-1 (final) ----
            if n_steps > 1:
                a_flip()
                mlp_and_q()
                q_flip()
                sort_stats()
            nc.vector.tensor_scalar(out_sb, mu, scalar1=TWO_PI, scalar2=None,
                                    op0=Alu.mult)
            nc.sync.dma_start(out=OUT.ap().rearrange("(g p) -> p g", p=P),
                              in_=out_sb)

    nc.compile()
    return nc


def host_prng(n_steps=ITERS - 1):
    """Exactly the reference's PRNG stream, on host CPU."""
    import jax
    import jax.numpy as jnp
    cpu = jax.devices("cpu")[0]
    with jax.default_device(cpu):
        # commit the key to CPU so the whole PRNG stream is computed by the
        # CPU backend bit-exactly (matching the reference harness)
        key = jax.device_put(jax.random.key(42), cpu)
        k0, kloop = jax.random.split(key)
        angles0 = np.asarray(jax.random.uniform(k0, (BATCH, M),
                                                dtype=jnp.float32))
        keys = jax.random.split(kloop, ITERS - 1)
        eps = np.stack([
            np.asarray(jax.random.normal(keys[t], (BATCH, M),
                                         dtype=jnp.float32))
            for t in range(max(n_steps - 1, 1))
        ])
    return angles0, eps


def make_in_map(core, states, W1, b1, W2, b2, W3, b3, angles0, eps):
    sl = slice(core * B, (core + 1) * B)
    S = np.ascontiguousarray(states[sl]).reshape(G, P, 2)
    xrep = np.ascontiguousarray(
        np.broadcast_to(S[:, :, None, :], (G, P, M, 2)).transpose(3, 0, 1, 2)
    ).reshape(2, N)
    a0 = np.ascontiguousarray(angles0[sl]).reshape(N)
    epsc = np.ascontiguousarray(
        eps[:, sl, :].reshape(-1, G, P, M).transpose(0, 2, 1, 3)
    ).reshape(-1, P, G * M)
    w3p = np.zeros((HIDDEN, 32), np.float16)
    w3p[:, 0] = W3[:, 0].astype(np.float16)
    return {
        "XREP": xrep.astype(np.float32),
        "A0": a0.astype(np.float32),
        "EPS": epsc.astype(np.float32),
        "W1D": W1.astype(np.float32),
        "W2D": W2.astype(np.float32),
        "W3D": w3p,  # fp16
        "B1D": b1.astype(np.float32),
        "B2D": b2.astype(np.float32),
        "B3D": np.full((P,), np.float32(b3[0]), np.float32),
    }


LAST_RESULTS = None


def kernel(states, W1, b1, W2, b2, W3, b3, _trace=False):
    global LAST_RESULTS
    from concourse.bass_utils import run_bass_kernel_spmd

    n_steps = ITERS - 1
    if n_steps not in _PROG_CACHE:
        _PROG_CACHE[n_steps] = build_program(n_steps)
    nc = _PROG_CACHE[n_steps]

    angles0, eps = host_prng(n_steps)
    in_maps = [
        make_in_map(c, states, W1, b1, W2, b2, W3, b3, angles0, eps)
        for c in range(NCORES)
    ]
    res = run_bass_kernel_spmd(nc, in_maps, core_ids=list(range(NCORES)),
                               trace=_trace)
    LAST_RESULTS = res
    out = np.concatenate([res.results[c]["OUT"] for c in range(NCORES)])
    return out.astype(np.float32)


# revision 9
# speedup vs baseline: 2.3819x; 1.0757x over previous
"""Trainium2 Bass kernel for nn_DQN CEM sampling problem.

Data-parallel over batch: 4096 rows -> 8 cores x 512 rows. Each core runs the
full 99-step CEM loop on its shard; the tiny MLP weights are replicated.

Per-core layout (B=512 rows as G=4 groups of P=128 partitions):
  - MLP runs feature-major: activations [feature, n] with n = (g, p, m) flattened,
    so each layer is a single tensor-engine pass (K on partitions).
  - q is produced batch-major-compatible via 4-way column-tiled matmuls
    (tile_position=(0,32g)), drained to SBUF, then "flipped" (partition<->free)
    with strided SBUF->SBUF DMAs into a [128, 4, 64] tile (cols 50:64 = -1e30 pad).
  - top-32 of 50 per row via a tie-exact bitonic network (sort both 32-halves
    descending, then one compare of half A vs reversed half B) on the vector
    engine; mean/var via bn_stats/bn_aggr; std = sqrt(var * 32/31) (ddof=1).
  - next angles = mu + std * eps sampled batch-major, flipped back into the
    feature-major x row via DMA.

All PRNG tensors (angles0, eps_t) are host-precomputed with the exact jax calls
the reference makes (key 42), laid out per-core, and streamed from DRAM.
"""

import numpy as np

BATCH = 4096
M = 50
NTOP = 32
ITERS = 100  # reference ITERS; device runs ITERS-1 = 99 qnet/stats steps
HIDDEN = 100
NCORES = 8
B = BATCH // NCORES  # 512 rows per core
G = 4                # partition groups per core
P = 128              # rows per group (partitions)
NPG = P * M          # columns per group = 6400
N = G * NPG          # columns per core = 25600
NEG = -1.0e30
TWO_PI = 6.283185307179586

_PROG_CACHE = {}


def _mlp_tiles():
    """(ti, w) matmul column tiles covering one group's NPG columns."""
    tiles = []
    off = 0
    while off < NPG:
        w = min(512, NPG - off)
        tiles.append((off, w))
        off += w
    return tiles


def build_program(n_steps=ITERS - 1):
    """Build the single-core Bass/Tile program (SPMD across cores).

    n_steps = number of qnet+stats iterations (reference does 99)."""
    import concourse.bacc as bacc
    import concourse.bass as bass
    import concourse.tile as tile
    import concourse.mybir as mybir

    f32 = mybir.dt.float32
    f32r = mybir.dt.float32r
    Alu = mybir.AluOpType
    Act = mybir.ActivationFunctionType

    nc = bacc.Bacc("TRN2", target_bir_lowering=False, debug=False)

    # DRAM I/O
    XREP = nc.dram_tensor("XREP", [2, N], f32r, kind="ExternalInput")
    A0 = nc.dram_tensor("A0", [N], f32r, kind="ExternalInput")
    EPS = nc.dram_tensor("EPS", [max(n_steps - 1, 1), P, G * M], f32,
                         kind="ExternalInput")
    W1D = nc.dram_tensor("W1D", [3, HIDDEN], f32r, kind="ExternalInput")
    W2D = nc.dram_tensor("W2D", [HIDDEN, HIDDEN], f32r, kind="ExternalInput")
    W3D = nc.dram_tensor("W3D", [HIDDEN, 32], mybir.dt.float16, kind="ExternalInput")
    B1D = nc.dram_tensor("B1D", [HIDDEN], f32, kind="ExternalInput")
    B2D = nc.dram_tensor("B2D", [HIDDEN], f32, kind="ExternalInput")
    B3D = nc.dram_tensor("B3D", [P], f32, kind="ExternalInput")
    OUT = nc.dram_tensor("OUT", [B], f32, kind="ExternalOutput")

    tiles = _mlp_tiles()

    with tile.TileContext(nc) as tc:
        with (
            tc.tile_pool(name="statics", bufs=1) as statics,
            tc.tile_pool(name="h1pool", bufs=6) as h1pool,
            tc.tile_pool(name="h2pool", bufs=6) as h2pool,
            tc.tile_pool(name="ps1", bufs=4, space=bass.MemorySpace.PSUM) as ps1,
            tc.tile_pool(name="ps2", bufs=3, space=bass.MemorySpace.PSUM) as ps2,
            tc.tile_pool(name="psq", bufs=1, space=bass.MemorySpace.PSUM) as psq,
        ):
            # --- static tiles ---
            x = statics.tile([3, N], f32r)             # fm input rows s0,s1,angle
            q_sb = statics.tile([P, NPG], f32)        # q rows at partitions 0/32/64/96
            q64 = statics.tile([P, G, 64], f32)       # batch-major q + pad
            SA = statics.tile([P, G * 64], f32)       # sort ping
            SB = statics.tile([P, G * 64], f32)       # sort pong
            top32 = statics.tile([P, G, NTOP], f32)
            bnst = statics.tile([P, G, 6], f32)
            mv = statics.tile([P, G, 2], f32)
            mu = statics.tile([P, G], f32)
            std = statics.tile([P, G], f32)
            a_bm = statics.tile([P, G, M], f32r)        # sampled angles, batch-major
            tmp_s = statics.tile([P, G, M], f32)
            eps_sb = statics.tile([P, G * M], f32)
            out_sb = statics.tile([P, G], f32)
            w1s = statics.tile([3, HIDDEN], f32r)
            w2s = statics.tile([HIDDEN, HIDDEN], f32r)
            w3s = statics.tile([HIDDEN, 32], mybir.dt.float16)
            b1s = statics.tile([HIDDEN, 1], f32)
            b2s = statics.tile([HIDDEN, 1], f32)
            b3s = statics.tile([P, 1], f32)

            # --- one-time setup ---
            nc.sync.dma_start(out=w1s, in_=W1D.ap())
            nc.sync.dma_start(out=w2s, in_=W2D.ap())
            nc.sync.dma_start(out=w3s, in_=W3D.ap())
            nc.sync.dma_start(out=b1s, in_=B1D.ap())
            nc.sync.dma_start(out=b2s, in_=B2D.ap())
            nc.sync.dma_start(out=b3s, in_=B3D.ap())
            nc.sync.dma_start(out=x[0:2, :], in_=XREP.ap())
            nc.vector.memset(q64[:, :, M:64], NEG)

            def mlp_and_q():
                """x row2 -> q_sb (all 4 groups), feature-major MLP.

                Matmuls are batched per weight (4 consecutive same-lhsT MMs)
                so LDWEIGHTS of MM i+1 pipelines under MM i."""
                for (off, w) in tiles:
                    h1ps, h1ss, h2ss = [], [], []
                    for g in range(G):
                        col = g * NPG + off
                        h1p = ps1.tile([HIDDEN, 512], f32, tag="h1p")
                        nc.tensor.matmul(h1p[:, :w], w1s, x[:, col:col + w])
                        h1ps.append(h1p)
                    for g in range(G):
                        h1s = h1pool.tile([HIDDEN, 512], f32r, tag="h1s")
                        nc.scalar.activation(h1s[:, :w], h1ps[g][:, :w],
                                             Act.Relu, bias=b1s, scale=1.0)
                        h1ss.append(h1s)
                    for g in range(G):
                        h2p = ps2.tile([HIDDEN, 512], f32, tag="h2p")
                        nc.tensor.matmul(h2p[:, :w], w2s, h1ss[g][:, :w])
                        h2s = h2pool.tile([HIDDEN, 512], mybir.dt.float16,
                                          tag="h2s")
                        nc.vector.tensor_scalar(h2s[:, :w], h2p[:, :w],
                                                scalar1=b2s, scalar2=0.0,
                                                op0=Alu.add, op1=Alu.max)
                        h2ss.append(h2s)
                    qp = psq.tile([P, 512], f32, tag="qp")
                    for g in range(G):
                        nc.tensor.matmul(qp[32 * g:32 * g + 32, :w], w3s,
                                         h2ss[g][:, :w],
                                         tile_position=(0, 32 * g))
                    # drain all 4 groups' q rows (plus zero rows) in one op
                    nc.scalar.copy(q_sb[:, off:off + w], qp[:, :w])

            def q_flip():
                for g in range(G):
                    nc.sync.dma_start(
                        out=q64[:, g, 0:M],
                        in_=q_sb[32 * g:32 * g + 1, :].rearrange(
                            "a (p m) -> a p m", m=M),
                    )

            def a_flip():
                for g in range(G):
                    nc.sync.dma_start(
                        out=x[2:3, g * NPG:(g + 1) * NPG].rearrange(
                            "a (p m) -> a p m", m=M),
                        in_=a_bm[:, g, :],
                    )

            def sort_stats():
                """q64 -> top32 -> mv (mean, var) -> mu (incl. b3)."""
                # halves view: [P, 8, 32] on each buffer
                def hv(t):
                    return t.rearrange("p (h m) -> p h m", m=32)

                src = hv(q64.rearrange("p g m -> p (g m)"))
                dst_list = [hv(SA), hv(SB)]
                which = 0
                for k in [2, 4, 8, 16, 32]:
                    # flip substage on blocks of k
                    dst = dst_list[which]; which ^= 1
                    nbk = 32 // k
                    s4 = src.rearrange("p h (nb k) -> p h nb k", k=k)
                    d4 = dst.rearrange("p h (nb k) -> p h nb k", k=k)
                    lo_in = s4[:, :, :, 0:k // 2]
                    hi_in = s4[:, :, :, k // 2:k]
                    lo_rev = s4[:, :, :, k // 2 - 1::-1] if k > 1 else None
                    hi_rev = s4[:, :, :, k - 1:k // 2 - 1:-1]
                    nc.vector.tensor_tensor(d4[:, :, :, 0:k // 2], lo_in,
                                            hi_rev, op=Alu.max)
                    nc.vector.tensor_tensor(d4[:, :, :, k // 2:k], hi_in,
                                            lo_rev, op=Alu.min)
                    src = dst
                    d = k // 4
                    while d >= 1:
                        dst = dst_list[which]; which ^= 1
                        s5 = src.rearrange("p h (nb two d) -> p h nb two d",
                                           two=2, d=d)
                        d5 = dst.rearrange("p h (nb two d) -> p h nb two d",
                                           two=2, d=d)
                        nc.vector.tensor_tensor(d5[:, :, :, 0, :],
                                                s5[:, :, :, 0, :],
                                                s5[:, :, :, 1, :], op=Alu.max)
                        nc.vector.tensor_tensor(d5[:, :, :, 1, :],
                                                s5[:, :, :, 0, :],
                                                s5[:, :, :, 1, :], op=Alu.min)
                        src = dst
                        d //= 2
                # merge: top32[p,g,i] = max(A[p,g,i], B[p,g,31-i])
                sg = src.rearrange("p (g h) m -> p g h m", g=G)
                nc.vector.tensor_tensor(top32, sg[:, :, 0, :],
                                        sg[:, :, 1, ::-1], op=Alu.max)
                for g in range(G):
                    nc.vector.bn_stats(bnst[:, g, :], top32[:, g, :])
                    nc.vector.bn_aggr(mv[:, g, :], bnst[:, g:g + 1, :])
                nc.vector.tensor_scalar(mu, mv[:, :, 0], scalar1=b3s,
                                        scalar2=None, op0=Alu.add)

            def sample(t):
                """a_bm = mu + std * eps[t]  (t is python int or loop var)."""
                nc.scalar.activation(std, mv[:, :, 1], Act.Sqrt,
                                     scale=float(NTOP) / (NTOP - 1))
                if isinstance(t, int):
                    eps_src = EPS.ap()[t - 1:t, :, :]
                else:
                    eps_src = EPS.ap()[bass.ds(t - 1, 1), :, :]
                nc.sync.dma_start(out=eps_sb, in_=eps_src)
                epsv = eps_sb.rearrange("p (g m) -> p g m", m=M)
                stdb = std.unsqueeze(2).to_broadcast((P, G, M))
                mub = mu.unsqueeze(2).to_broadcast((P, G, M))
                nc.vector.tensor_tensor(tmp_s, epsv, stdb, op=Alu.mult)
                nc.vector.tensor_tensor(a_bm, tmp_s, mub, op=Alu.add)

            # ---- t = 0 ----
            nc.sync.dma_start(out=x[2:3, :], in_=A0.ap().rearrange("(a n) -> a n", a=1))
            mlp_and_q()
            q_flip()
            sort_stats()
            if n_steps > 1:
                sample(1)

            # ---- t = 1 .. n_steps-2 (dynamic loop) ----
            if n_steps > 2:
                with tc.For_i(1, n_steps - 1) as it:
                    a_flip()
                    mlp_and_q()
                    q_flip()
                    sort_stats()
                    sample(it + 1)

            # ---- t = n_steps-1 (final) ----
            if n_steps > 1:
                a_flip()
                mlp_and_q()
                q_flip()
                sort_stats()
            nc.vector.tensor_scalar(out_sb, mu, scalar1=TWO_PI, scalar2=None,
                                    op0=Alu.mult)
            nc.sync.dma_start(out=OUT.ap().rearrange("(g p) -> p g", p=P),
                              in_=out_sb)

    nc.compile()
    return nc


def host_prng(n_steps=ITERS - 1):
    """Exactly the reference's PRNG stream, on host CPU."""
    import jax
    import jax.numpy as jnp
    cpu = jax.devices("cpu")[0]
    with jax.default_device(cpu):
        # commit the key to CPU so the whole PRNG stream is computed by the
        # CPU backend bit-exactly (matching the reference harness)
        key = jax.device_put(jax.random.key(42), cpu)
        k0, kloop = jax.random.split(key)
        angles0 = np.asarray(jax.random.uniform(k0, (BATCH, M),
                                                dtype=jnp.float32))
        keys = jax.random.split(kloop, ITERS - 1)
        eps = np.stack([
            np.asarray(jax.random.normal(keys[t], (BATCH, M),
                                         dtype=jnp.float32))
            for t in range(max(n_steps - 1, 1))
        ])
    return angles0, eps


def make_in_map(core, states, W1, b1, W2, b2, W3, b3, angles0, eps):
    sl = slice(core * B, (core + 1) * B)
    S = np.ascontiguousarray(states[sl]).reshape(G, P, 2)
    xrep = np.ascontiguousarray(
        np.broadcast_to(S[:, :, None, :], (G, P, M, 2)).transpose(3, 0, 1, 2)
    ).reshape(2, N)
    a0 = np.ascontiguousarray(angles0[sl]).reshape(N)
    epsc = np.ascontiguousarray(
        eps[:, sl, :].reshape(-1, G, P, M).transpose(0, 2, 1, 3)
    ).reshape(-1, P, G * M)
    w3p = np.zeros((HIDDEN, 32), np.float16)
    w3p[:, 0] = W3[:, 0].astype(np.float16)
    return {
        "XREP": xrep.astype(np.float32),
        "A0": a0.astype(np.float32),
        "EPS": epsc.astype(np.float32),
        "W1D": W1.astype(np.float32),
        "W2D": W2.astype(np.float32),
        "W3D": w3p,  # fp16
        "B1D": b1.astype(np.float32),
        "B2D": b2.astype(np.float32),
        "B3D": np.full((P,), np.float32(b3[0]), np.float32),
    }


LAST_RESULTS = None


def kernel(states, W1, b1, W2, b2, W3, b3, _trace=False):
    global LAST_RESULTS
    from concourse.bass_utils import run_bass_kernel_spmd

    n_steps = ITERS - 1
    if n_steps not in _PROG_CACHE:
        _PROG_CACHE[n_steps] = build_program(n_steps)
    nc = _PROG_CACHE[n_steps]

    angles0, eps = host_prng(n_steps)
    in_maps = [
        make_in_map(c, states, W1, b1, W2, b2, W3, b3, angles0, eps)
        for c in range(NCORES)
    ]
    res = run_bass_kernel_spmd(nc, in_maps, core_ids=list(range(NCORES)),
                               trace=_trace)
    LAST_RESULTS = res
    out = np.concatenate([res.results[c]["OUT"] for c in range(NCORES)])
    return out.astype(np.float32)
